# revision 4
# baseline (speedup 1.0000x reference)
"""GCN2Conv (variant=False) Trainium2 kernel — v2.

out = beta * (support @ theta) + (1-beta) * support
support = (1-alpha) * (D^-1/2 (A+I) D^-1/2 @ x) + alpha * h0
beta = log(lamda/l + 1)

Sharding: B=4 graphs over 8 cores -> 2 cores per graph, each owning
m_rows = N/2 = 1500 adjacency rows (global column order everywhere).

v2 device pipeline per core (vs v1: loads were HWDGE and serialized with
the xbar transposes on one ring; degree tail was ~35us serial):
  Phase 1: 12 per-m-tile SWDGE cast-loads (fp32 HBM -> bf16 SBUF, casting
    inside the DMA, off the HWDGE rings). Per tile: DVE rowsum
    (tensor_reduce) into a [128, MT] degree column; one blocked xbar
    transpose into SBUF-resident A^T, ALTERNATING between the two HWDGE
    rings (sync + scalar) so two transposes drain concurrently.
    Meanwhile PE transposes x_loc/h0_loc tiles for the epilogue.
  Tail: dis = rsqrt(deg+1) computed in the [128, MT] column layout (no
    single-partition vector ops), PE-transposed + stored to DRAM; pair
    AllGather of the 1500-float dis vector (gated on the last xbar --
    xbar || collective is a HW deadlock); row-broadcast scale vectors and
    qT prep overlap the collective, as do PE warm-up matmuls.
  Phase 2: xs = dis_k * x in bf16; per 512-wide m-chunk, 24 k-block
    matmuls accumulate hi^T in PSUM; epilogue (rs*hi + qT, theta matmul,
    (1-b)*sup + o2, PE transpose back) is software-pipelined one chunk
    behind the matmuls so PE never stalls on DVE.
"""

import math
import sys

import numpy as np

sys.path.insert(0, "/opt/trn_rl_repo")

import concourse.bacc as bacc
import concourse.mybir as mybir
import concourse.tile as tile
from concourse import bass_utils, masks
from concourse.mybir import dt

AF = mybir.ActivationFunctionType

F = 128          # feature dim (= theta size), fixed
P = 128          # SBUF partitions
CHUNK = 512      # phase-2 m-chunk width (one fp32 PSUM bank)

B_FULL, N_FULL = 4, 3000
N_CORES_FULL = 8
M_FULL = N_FULL // 2

STREAM_BUFS = 3  # adj bf16 stream tiles in flight


def _tile_sizes(total, step):
    return [min(step, total - s) for s in range(0, total, step)]


def build_program(n_nodes, m_rows, n_cores, alpha, beta, at_dtype=dt.bfloat16,
                  debug_dump=False):
    """Build the SPMD Bass program (identical on every core).

    Per-core external inputs (host pre-slices):
      adj_rows [m_rows, n_nodes], x_full [n_nodes, F], x_loc [m_rows, F],
      h0_loc [m_rows, F], theta [F, F].
    Output: out [m_rows, F].
    Cores 2g, 2g+1 own rows [0:m_rows], [m_rows:2*m_rows] of graph g.
    """
    assert n_nodes == 2 * m_rows
    c1 = 1.0 - alpha

    KT = math.ceil(n_nodes / P)        # k blocks (adj cols / nodes)
    kw = _tile_sizes(n_nodes, P)
    MT = math.ceil(m_rows / P)         # local m tiles
    mh = _tile_sizes(m_rows, P)
    mfull, mtail = m_rows // P, m_rows % P
    kfull, ktail = n_nodes // P, n_nodes % P
    KTP = KT * P
    # phase-2 chunks: groups of up to 4 full m-tiles (512 cols) or the tail
    # tile alone -- each chunk is one contiguous piece of A^T and gets its
    # own PSUM accumulation bank
    mchunks = []
    ti = 0
    while ti < MT:
        if mh[ti] == P:
            tj = ti
            while tj < MT and mh[tj] == P and tj - ti < 4:
                tj += 1
            mchunks.append((ti * P, (tj - ti) * P, ti, tj, P))
            ti = tj
        else:
            mchunks.append((ti * P, mh[ti], ti, ti + 1, mh[ti]))
            ti += 1

    nc = bacc.Bacc(
        "TRN2", target_bir_lowering=False, debug=False, num_devices=n_cores
    )
    adj = nc.dram_tensor("adj_rows", [m_rows, n_nodes], dt.float32, kind="ExternalInput")
    x_full = nc.dram_tensor("x_full", [n_nodes, F], dt.float32, kind="ExternalInput")
    x_loc = nc.dram_tensor("x_loc", [m_rows, F], dt.float32, kind="ExternalInput")
    h0_loc = nc.dram_tensor("h0_loc", [m_rows, F], dt.float32, kind="ExternalInput")
    theta = nc.dram_tensor("theta", [F, F], dt.float32, kind="ExternalInput")
    out_d = nc.dram_tensor("out", [m_rows, F], dt.float32, kind="ExternalOutput")

    groups = [[2 * g, 2 * g + 1] for g in range(n_cores // 2)]

    with tile.TileContext(nc) as tc:
        from contextlib import ExitStack
        from concourse.tile import add_dep_helper as _adh

        with ExitStack() as ctx:
            ep = ctx.enter_context

            consts = ep(tc.tile_pool(name="consts", bufs=1))
            at_pool = ep(tc.tile_pool(name="at", bufs=1))
            stream_pool = ep(tc.tile_pool(name="stream", bufs=STREAM_BUFS))
            deg_pool = ep(tc.tile_pool(name="deg", bufs=1))
            xs_pool = ep(tc.tile_pool(name="xs", bufs=1))
            tvec_pool = ep(tc.tile_pool(name="tvec", bufs=1))
            sup_pool = ep(tc.tile_pool(name="sup", bufs=2))
            outc_pool = ep(tc.tile_pool(name="outc", bufs=2))
            outt_pool = ep(tc.tile_pool(name="outt", bufs=1))
            ptx_pool = ep(tc.tile_pool(name="ptx", bufs=2, space="PSUM"))
            dram = ep(tc.tile_pool(name="dram", bufs=1, space="DRAM"))

            ident = consts.tile([P, P], dt.float32)
            masks.make_identity(nc, ident[:])

            theta_sb = consts.tile([F, F], dt.float32)
            nc.sync.dma_start(theta_sb[:], theta[:])
            thetaB = consts.tile([F, F], dt.float32)
            nc.vector.tensor_scalar_mul(thetaB[:], theta_sb[:], beta)

            # x in [k_local, (kb, f)] layout, cast to bf16 during the DMA
            xg = xs_pool.tile([P, KT * F], at_dtype)
            if kfull:
                nc.gpsimd.dma_start(
                    xg[:].rearrange("p (kb f) -> p kb f", kb=KT)[:, 0:kfull, :],
                    x_full[0 : kfull * P, :].rearrange("(kb p) f -> p kb f", p=P),
                )
            if ktail:
                nc.gpsimd.dma_start(
                    xg[0:ktail, kfull * F : (kfull + 1) * F],
                    x_full[kfull * P : n_nodes, :],
                )

            # x_loc / h0_loc bulk loads in per-m-tile [p, (i f)] layout
            xn_all = xs_pool.tile([P, MT * F], dt.float32, tag="xn_all")
            hn_all = xs_pool.tile([P, MT * F], dt.float32, tag="hn_all")
            for src, dst in ((x_loc, xn_all), (h0_loc, hn_all)):
                if mfull:
                    nc.gpsimd.dma_start(
                        dst[:].rearrange("p (i f) -> p i f", i=MT)[:, 0:mfull, :],
                        src[0 : mfull * P, :].rearrange("(i p) f -> p i f", p=P),
                    )
                if mtail:
                    nc.gpsimd.dma_start(
                        dst[0:mtail, mfull * F : (mfull + 1) * F],
                        src[mfull * P : m_rows, :],
                    )

            # A^T resident in SBUF: [k_local, (m_tile, kb, m_local)] -- one
            # contiguous [KT, 128] region per m-tile so the blocked xbar
            # transpose writes it in a single instruction
            AT = at_pool.tile([P, MT * KTP], at_dtype)
            AT4 = AT[:].rearrange("p (i kb m) -> p i kb m", i=MT, kb=KT)

            # local degree columns: col i = row sums of local m-tile i
            deg_col = deg_pool.tile([P, MT], dt.float32)
            nc.gpsimd.memset(deg_col[:], 0.0)  # garbage lanes stay rsqrt-safe

            # ---------------- Phase 1: stream adj, rowsum + transpose ----------
            # SWDGE cast-loads fp32 -> bf16 (half the SBUF writes, no ACT
            # cast); DVE rowsums; xbar transposes alternate HWDGE rings.
            # The xbar-transpose's data accesses are invisible to Tile's dep
            # tracker, so fence manually: RAW load->transpose, WAR transpose->
            # load on stream-buffer reuse.
            t_insts = []
            for i in range(MT):
                h = mh[i]
                nat = stream_pool.tile([P, KTP], at_dtype, tag="nat")
                ld = nc.gpsimd.dma_start(
                    nat[:h, 0:n_nodes], adj[P * i : P * i + h, :]
                )
                if i >= STREAM_BUFS:
                    _adh(ld.ins, t_insts[i - STREAM_BUFS].ins, sync=True,
                         reason="nat slot WAR vs xbar transpose")
                nc.vector.tensor_reduce(
                    deg_col[:h, i : i + 1], nat[:h, 0:n_nodes],
                    mybir.AxisListType.X, mybir.AluOpType.add,
                )
                # one blocked transpose for the whole row-tile:
                # in [128, KT*128] -> out [128, KT, 128] (3D out folds kb into
                # the logical partition dim). Tail tiles read/write garbage
                # rows beyond h -- never consumed.
                t_eng = nc.sync if (i % 2 == 0) else nc.scalar
                t_inst = t_eng.dma_start_transpose(
                    AT4[:, i, :, :], nat[:P, 0:KTP]
                )
                _adh(t_inst.ins, ld.ins, sync=True,
                     reason="xbar transpose RAW fence on cast-load")
                t_insts.append(t_inst)

            # x_loc / h0_loc transposed: xT [f, m], h0aT = alpha * h0^T
            # (PE transposes run during the load stream; PE is otherwise idle)
            xT = xs_pool.tile([P, m_rows], dt.float32, tag="xT")
            h0aT = xs_pool.tile([P, m_rows], dt.float32, tag="h0aT")
            for i in range(MT):
                h = mh[i]
                xt_ps = ptx_pool.tile([P, P], dt.float32, tag="sm")
                nc.tensor.transpose(
                    xt_ps[:F, :h], xn_all[:h, i * F : i * F + F], ident[:h, :h]
                )
                nc.vector.tensor_copy(xT[:, P * i : P * i + h], xt_ps[:F, :h])

                ht_ps = ptx_pool.tile([P, P], dt.float32, tag="sm")
                nc.tensor.transpose(
                    ht_ps[:F, :h], hn_all[:h, i * F : i * F + F], ident[:h, :h]
                )
                nc.scalar.activation(
                    h0aT[:, P * i : P * i + h], ht_ps[:F, :h], AF.Copy, scale=alpha
                )

            # ---------------- degree tail: dis = rsqrt(deg+1), exchange -------
            degp = deg_pool.tile([P, MT], dt.float32, tag="degp")
            nc.vector.tensor_scalar_add(degp[:], deg_col[:], 1.0)
            dis_col = deg_pool.tile([P, MT], dt.float32, tag="dis_col")
            nc.vector.reciprocal(dis_col[:], degp[:])
            nc.scalar.sqrt(dis_col[:], dis_col[:])

            degT_ps = ptx_pool.tile([P, P], dt.float32, tag="sm")
            nc.tensor.transpose(degT_ps[:MT, :P], dis_col[:P, :MT], ident[:P, :P])
            disT = deg_pool.tile([P, P], dt.float32, tag="disT")
            nc.vector.tensor_copy(disT[:MT, :P], degT_ps[:MT, :P])

            dis_loc_d = dram.tile([m_rows], dt.float32)
            dis_full_d = dram.tile([n_nodes], dt.float32)
            st_insts = []
            if mfull:
                st_insts.append(nc.gpsimd.dma_start(
                    dis_loc_d[0 : mfull * P].rearrange("(a b) -> a b", b=P),
                    disT[0:mfull, :],
                ))
            if mtail:
                st_insts.append(nc.gpsimd.dma_start(
                    dis_loc_d[mfull * P : m_rows].rearrange("(a b) -> a b", a=1),
                    disT[mfull : mfull + 1, 0:mtail],
                ))
            # xbar-mode transposes must not run concurrently with the
            # collective's DMAs (HW deadlock) -- gate on BOTH rings' last
            # transposes.
            ag = nc.gpsimd.collective_compute(
                "AllGather",
                mybir.AluOpType.bypass,
                replica_groups=groups,
                ins=[dis_loc_d[:]],
                outs=[dis_full_d[:]],
            )
            for t in t_insts[-2:]:
                _adh(ag.ins, t.ins, sync=True,
                     reason="xbar-vs-collective serialization")

            # PE HAM warm-up: dummy matmuls during the collective so phase-2
            # matmuls start at the warm 2.4 GHz clock
            with tc.tile_pool(name="warm_ps", bufs=1, space="PSUM") as warm_pool:
                wp = warm_pool.tile([P, CHUNK], dt.float32)
                n_warm = 16
                wfree = min(CHUNK, KT * F)
                for j in range(n_warm):
                    wmm = nc.tensor.matmul(
                        wp[:P, 0:wfree],
                        xg[:P, 0:P],
                        xg[:P, 0:wfree],
                        start=(j == 0),
                        stop=(j == n_warm - 1),
                    )
                    if j == 0:
                        _adh(wmm.ins, st_insts[0].ins, sync=True,
                             reason="warmup during collective")

            # row-layout scale vectors (need only LOCAL dis -> overlap AG):
            # reload [1, m] row from DRAM, broadcast across partitions
            vecs = tvec_pool.tile([P, m_rows], dt.float32)
            dis_row = vecs[0:1, :]
            ld_row = nc.gpsimd.dma_start(
                dis_row[0:1, 0:m_rows],
                dis_loc_d[:].rearrange("(a b) -> a b", a=1),
            )
            s1_b = tvec_pool.tile([P, m_rows], dt.float32, tag="s1_b")
            nc.gpsimd.partition_broadcast(s1_b[:], dis_row)
            rs_b = tvec_pool.tile([P, m_rows], dt.float32, tag="rs_b")
            nc.vector.tensor_scalar_mul(rs_b[:], s1_b[:], c1)
            nc.vector.tensor_mul(s1_b[:], s1_b[:], rs_b[:])

            # qT = s1 * x^T + alpha * h0^T  (everything but the hi term)
            qT = xs_pool.tile([P, m_rows], dt.float32, tag="qT")
            nc.vector.tensor_mul(qT[:], xT[:], s1_b[:])
            nc.vector.tensor_add(qT[:], qT[:], h0aT[:])

            # global dis -> per k-block column layout [P, KT]
            dgT = deg_pool.tile([P, P], dt.float32, tag="dgT")
            nc.gpsimd.memset(dgT[:KT, :], 1.0)
            if kfull:
                nc.gpsimd.dma_start(
                    dgT[0:kfull, 0:P],
                    dis_full_d[0 : kfull * P].rearrange("(a b) -> a b", b=P),
                )
            if ktail:
                nc.gpsimd.dma_start(
                    dgT[kfull : kfull + 1, 0:ktail],
                    dis_full_d[kfull * P : n_nodes],
                )
            dg_ps = ptx_pool.tile([P, P], dt.float32, tag="sm")
            nc.tensor.transpose(dg_ps[:P, :KT], dgT[:KT, :P], ident[:KT, :KT])
            disg = deg_pool.tile([P, KT], dt.float32, tag="disg")
            nc.vector.tensor_copy(disg[:], dg_ps[:P, :KT])

            # xs = D^-1/2 x in [k_local, (kb, f)] bf16
            xs = xs_pool.tile([P, KT * F], at_dtype, tag="xs")
            for kb in range(KT):
                w = kw[kb]
                nc.vector.tensor_scalar_mul(
                    xs[:w, kb * F : kb * F + F],
                    xg[:w, kb * F : kb * F + F],
                    disg[:w, kb : kb + 1],
                )

            # ---------------- Phase 2: matmuls + epilogue ----------------------
            # software-pipelined: chunk c's 24 matmuls are emitted BEFORE
            # chunk c-1's epilogue so the in-order PE queue never stalls on
            # DVE results.
            out_sb = xs_pool.tile([P, MT * F], dt.float32, tag="out_sb")
            with tc.tile_pool(name="hi_ps", bufs=2, space="PSUM") as hi_pool, \
                 tc.tile_pool(name="o2_ps", bufs=2, space="PSUM") as o2_pool:
                hi_tiles = []

                def emit_mms(ci):
                    s, wc, ia, ib, tw = mchunks[ci]
                    hiT = hi_pool.tile([P, CHUNK], dt.float32)
                    for kb in range(KT):
                        w = kw[kb]
                        nc.tensor.matmul(
                            hiT[:F, 0:wc],
                            xs[:w, kb * F : kb * F + F],
                            AT4[:w, ia:ib, kb, 0:tw],
                            start=(kb == 0),
                            stop=(kb == KT - 1),
                        )
                    hi_tiles.append(hiT)

                def emit_epilogue(ci):
                    s, wc, ia, ib, tw = mchunks[ci]
                    hiT = hi_tiles[ci]
                    supT = sup_pool.tile([P, CHUNK], dt.float32)
                    nc.vector.tensor_mul(
                        supT[:, 0:wc], hiT[:F, 0:wc], rs_b[:, s : s + wc]
                    )
                    nc.vector.tensor_add(
                        supT[:, 0:wc], supT[:, 0:wc], qT[:, s : s + wc]
                    )
                    o2T = o2_pool.tile([P, CHUNK], dt.float32)
                    nc.tensor.matmul(
                        o2T[:F, 0:wc], thetaB[:F, :F], supT[:F, 0:wc],
                        start=True, stop=True,
                    )
                    outT = outc_pool.tile([P, CHUNK], dt.float32)
                    nc.vector.scalar_tensor_tensor(
                        outT[:, 0:wc], supT[:, 0:wc], 1.0 - beta, o2T[:F, 0:wc],
                        mybir.AluOpType.mult, mybir.AluOpType.add,
                    )
                    # back to natural [m, f] into the staging tile
                    for off in range(0, wc, P):
                        hh = min(P, wc - off)
                        ti_ = ia + off // P
                        ot_ps = ptx_pool.tile([P, P], dt.float32, tag="sm")
                        nc.tensor.transpose(
                            ot_ps[:hh, :F], outT[:F, off : off + hh], ident[:F, :F]
                        )
                        nc.vector.tensor_copy(
                            out_sb[:hh, ti_ * F : ti_ * F + F], ot_ps[:hh, :F]
                        )
                    # store this chunk's rows
                    if tw == P:
                        nc.gpsimd.dma_start(
                            out_d[s : s + wc, :].rearrange("(i p) f -> p i f", p=P),
                            out_sb[:].rearrange("p (i f) -> p i f", i=MT)[:, ia:ib, :],
                        )
                    else:
                        nc.gpsimd.dma_start(
                            out_d[s : s + wc, :], out_sb[0:wc, ia * F : ia * F + F]
                        )

                emit_mms(0)
                for ci in range(1, len(mchunks)):
                    emit_mms(ci)
                    emit_epilogue(ci - 1)
                emit_epilogue(len(mchunks) - 1)

    nc.compile()
    return nc


def make_in_maps(x, adj, h0, theta, n_cores):
    m = x.shape[1] // 2
    in_maps = []
    for c in range(n_cores):
        b, half = c // 2, c % 2
        r0 = half * m
        in_maps.append(
            {
                "adj_rows": adj[b, r0 : r0 + m, :],
                "x_full": x[b],
                "x_loc": x[b, r0 : r0 + m, :],
                "h0_loc": h0[b, r0 : r0 + m, :],
                "theta": theta,
            }
        )
    return in_maps


_CACHE = {}


def _get_program(key, *args, **kwargs):
    if key not in _CACHE:
        _CACHE[key] = build_program(*args, **kwargs)
    return _CACHE[key]


def kernel(x, adj, h0, theta, lamda, alpha, l):
    x = np.asarray(x, dtype=np.float32)
    adj = np.asarray(adj, dtype=np.float32)
    h0 = np.asarray(h0, dtype=np.float32)
    theta = np.asarray(theta, dtype=np.float32)
    lamda_f = float(np.asarray(lamda))
    alpha_f = float(np.asarray(alpha))
    l_f = float(np.asarray(l))
    beta_f = float(math.log(lamda_f / l_f + 1.0))

    B, N, Fdim = x.shape
    assert (B, N, Fdim) == (B_FULL, N_FULL, F)
    M = N // 2

    nc = _get_program(
        ("full", alpha_f, beta_f), N, M, N_CORES_FULL, alpha_f, beta_f
    )

    in_maps = make_in_maps(x, adj, h0, theta, N_CORES_FULL)
    res = bass_utils.run_bass_kernel_spmd(
        nc, in_maps, list(range(N_CORES_FULL))
    ).results

    out = np.empty((B, N, Fdim), dtype=np.float32)
    for c in range(N_CORES_FULL):
        b, half = c // 2, c % 2
        out[b, half * M : (half + 1) * M, :] = res[c]["out"]
    return out


# revision 19
# speedup vs baseline: 1.6221x; 1.6221x over previous
"""GCN2Conv (variant=False) Trainium2 kernel — v3.

out = beta * (support @ theta) + (1-beta) * support
support = (1-alpha) * (D^-1/2 (A+I) D^-1/2 @ x) + alpha * h0
beta = log(lamda/l + 1)

Sharding: B=4 graphs over 8 cores -> 2 cores per graph, each owning
m_rows = N/2 = 1500 adjacency rows (global column order everywhere).

Key lessons baked in (from v1/v2 traces):
  - Tile recycles 8 DMA-completion semaphore lanes round-robin PER DGE
    CLASS, and engine queues are strict FIFO: a sem-wait parked in front
    of a DMA stalls everything behind it. So the SWDGE queue carries ONLY
    the 12 adj cast-loads (back-to-back, no interleaved small DMAs), all
    other loads ride the two HWDGE rings, and the stream ring is 6 tiles
    deep so the WAR fence (load i vs xbar transpose i-6) never binds.
  - fp32->bf16 cast happens inside the SWDGE DMA (wire-rate ~26 GB/s per
    engine x16, measured) -- no ACT cast pass at all.
  - xbar transposes alternate between the sync and scalar HWDGE rings so
    two drain concurrently; collectives are fenced after the last two.
  - The degree tail runs in [128, MT] column layout; the row-broadcast of
    dis uses PE ones-matmuls (NOT gpsimd partition_broadcast, which would
    park behind the AllGather in the gpsimd FIFO) so qT/rs_b/s1_b prep
    overlaps the collective.
  - Phase-2: xs scaling (DVE) is emitted interleaved ahead of the chunk
    matmuls so PE chases the scale stream; epilogue is software-pipelined
    one chunk behind the matmuls.
"""

import math
import sys

import numpy as np

sys.path.insert(0, "/opt/trn_rl_repo")

import concourse.bacc as bacc
import concourse.mybir as mybir
import concourse.tile as tile
from concourse import bass_utils, masks
from concourse.mybir import dt

AF = mybir.ActivationFunctionType

F = 128          # feature dim (= theta size), fixed
P = 128          # SBUF partitions
CHUNK = 512      # phase-2 m-chunk width (one fp32 PSUM bank)

B_FULL, N_FULL = 4, 3000
N_CORES_FULL = 8
M_FULL = N_FULL // 2

RING_PAIRS = 3   # adj bf16 stream pair-tiles in flight
N_WARM = 24      # PE warm-up matmuls spanning the collective


def _tile_sizes(total, step):
    return [min(step, total - s) for s in range(0, total, step)]


def build_program(n_nodes, m_rows, n_cores, alpha, beta, at_dtype=dt.bfloat16,
                  debug_dump=False):
    """Build the SPMD Bass program (identical on every core).

    Per-core external inputs (host pre-slices):
      adj_rows [m_rows, n_nodes], x_full [n_nodes, F], x_loc [m_rows, F],
      h0_loc [m_rows, F], theta [F, F].
    Output: out [m_rows, F].
    Cores 2g, 2g+1 own rows [0:m_rows], [m_rows:2*m_rows] of graph g.
    """
    assert n_nodes == 2 * m_rows
    c1 = 1.0 - alpha

    KT = math.ceil(n_nodes / P)        # k blocks (adj cols / nodes)
    kw = _tile_sizes(n_nodes, P)
    MT = math.ceil(m_rows / P)         # local m tiles
    mh = _tile_sizes(m_rows, P)
    mfull, mtail = m_rows // P, m_rows % P
    kfull, ktail = n_nodes // P, n_nodes % P
    KTP = KT * P
    # phase-2 chunks: groups of up to 4 full m-tiles (512 cols) or the tail
    # tile alone -- each chunk is one contiguous piece of A^T and gets its
    # own PSUM accumulation bank
    mchunks = []
    ti = 0
    while ti < MT:
        if mh[ti] == P:
            tj = ti
            while tj < MT and mh[tj] == P and tj - ti < 4:
                tj += 1
            mchunks.append((ti * P, (tj - ti) * P, ti, tj, P))
            ti = tj
        else:
            mchunks.append((ti * P, mh[ti], ti, ti + 1, mh[ti]))
            ti += 1

    nc = bacc.Bacc(
        "TRN2", target_bir_lowering=False, debug=False, num_devices=n_cores
    )
    adj = nc.dram_tensor("adj_rows", [m_rows, n_nodes], dt.float32, kind="ExternalInput")
    x_full = nc.dram_tensor("x_full", [n_nodes, F], dt.float32, kind="ExternalInput")
    x_loc = nc.dram_tensor("x_loc", [m_rows, F], dt.float32, kind="ExternalInput")
    h0_loc = nc.dram_tensor("h0_loc", [m_rows, F], dt.float32, kind="ExternalInput")
    theta = nc.dram_tensor("theta", [F, F], dt.float32, kind="ExternalInput")
    out_d = nc.dram_tensor("out", [m_rows, F], dt.float32, kind="ExternalOutput")

    groups = [[2 * g, 2 * g + 1] for g in range(n_cores // 2)]

    with tile.TileContext(nc) as tc:
        from contextlib import ExitStack
        from concourse.tile import add_dep_helper as _adh

        with ExitStack() as ctx:
            ep = ctx.enter_context

            consts = ep(tc.tile_pool(name="consts", bufs=1))
            at_pool = ep(tc.tile_pool(name="at", bufs=1))
            stream_pool = ep(tc.tile_pool(name="stream", bufs=RING_PAIRS))
            deg_pool = ep(tc.tile_pool(name="deg", bufs=1))
            xs_pool = ep(tc.tile_pool(name="xs", bufs=1))
            tvec_pool = ep(tc.tile_pool(name="tvec", bufs=1))
            sup_pool = ep(tc.tile_pool(name="sup", bufs=2))
            outc_pool = ep(tc.tile_pool(name="outc", bufs=2))
            ptx_pool = ep(tc.tile_pool(name="ptx", bufs=2, space="PSUM"))
            dram = ep(tc.tile_pool(name="dram", bufs=1, space="DRAM"))

            ident = consts.tile([P, P], dt.float32)
            masks.make_identity(nc, ident[:])
            identB = consts.tile([P, P], at_dtype)
            masks.make_identity(nc, identB[:])

            ones1 = consts.tile([1, P], dt.float32)
            nc.gpsimd.memset(ones1[:], 1.0)

            # local degree columns: col i = row sums of local m-tile i
            deg_col = deg_pool.tile([P, MT], dt.float32)
            nc.gpsimd.memset(deg_col[:], 0.0)  # garbage lanes stay rsqrt-safe
            dgT = deg_pool.tile([P, P], dt.float32, tag="dgT")
            nc.gpsimd.memset(dgT[:KT, :], 1.0)

            # non-adj loads ride the HWDGE rings (keep the SWDGE queue clean
            # for the adj cast-loads)
            theta_sb = consts.tile([F, F], dt.float32)
            nc.sync.dma_start(theta_sb[:], theta[:])
            thetaB = consts.tile([F, F], dt.float32)
            nc.vector.tensor_scalar_mul(thetaB[:], theta_sb[:], beta)

            # x in [k_local, (kb, f)] fp32 (cast to bf16 happens in the
            # phase-2 dis-scaling op)
            xg = xs_pool.tile([P, KT * F], dt.float32)
            if kfull:
                nc.sync.dma_start(
                    xg[:].rearrange("p (kb f) -> p kb f", kb=KT)[:, 0:kfull, :],
                    x_full[0 : kfull * P, :].rearrange("(kb p) f -> p kb f", p=P),
                )
            if ktail:
                nc.sync.dma_start(
                    xg[0:ktail, kfull * F : (kfull + 1) * F],
                    x_full[kfull * P : n_nodes, :],
                )

            # x_loc / h0_loc bulk loads in per-m-tile [p, (i f)] layout
            xn_all = xs_pool.tile([P, MT * F], dt.float32, tag="xn_all")
            hn_all = xs_pool.tile([P, MT * F], dt.float32, tag="hn_all")
            for src, dst in ((x_loc, xn_all), (h0_loc, hn_all)):
                if mfull:
                    nc.scalar.dma_start(
                        dst[:].rearrange("p (i f) -> p i f", i=MT)[:, 0:mfull, :],
                        src[0 : mfull * P, :].rearrange("(i p) f -> p i f", p=P),
                    )
                if mtail:
                    nc.scalar.dma_start(
                        dst[0:mtail, mfull * F : (mfull + 1) * F],
                        src[mfull * P : m_rows, :],
                    )

            # A^T resident in SBUF: [k_local, (m_tile, kb, m_local)] -- one
            # contiguous [KT, 128] region per m-tile so the blocked xbar
            # transpose writes it in a single instruction
            AT = at_pool.tile([P, MT * KTP], at_dtype)
            AT4 = AT[:].rearrange("p (i kb m) -> p i kb m", i=MT, kb=KT)

            xT = xs_pool.tile([P, m_rows], dt.float32, tag="xT")
            h0aT = xs_pool.tile([P, m_rows], dt.float32, tag="h0aT")

            # ---------------- Phase 1: stream adj, rowsum + transpose ----------
            # PAIR granularity loads: one SWDGE cast-load (fp32->bf16) covers
            # two m-tiles.  NO xbar: the DMA-transpose corrupts data when two
            # of them overlap on the two HWDGE rings, and Tile serializes
            # every SWDGE DMA against in-flight xbars anyway.  Instead the
            # (otherwise idle) PE transposes A in [128,128] blocks, 4 blocks
            # into one PSUM bank, and DVE drains each bank with a single
            # 512-wide copy into bf16 A^T.  ACT does the rowsums (Copy with
            # accum_out into a throwaway scratch so the PE transposes never
            # serialize behind it).  Every access is visible to Tile's dep
            # tracker -- no manual fences anywhere in the pipeline.
            pairs = [(j, list(range(2 * j, min(2 * j + 2, MT))))
                     for j in range((MT + 1) // 2)]
            KB_GRP = 4              # transposed blocks per PSUM bank drain
            red_scr = xs_pool.tile([P, KTP], at_dtype, tag="red_scr")
            bank_ctx = tc.tile_pool(name="bank_ps", bufs=4, space="PSUM")
            bank_pool = bank_ctx.__enter__()
            for j, tiles in pairs:
                nat = stream_pool.tile([P, 2 * KTP], at_dtype, tag="nat")
                nat2 = nat[:].rearrange("p (t k) -> p t k", t=2)
                if len(tiles) == 2 and mh[tiles[0]] == P and mh[tiles[1]] == P:
                    nc.gpsimd.dma_start(
                        nat2[:, :, 0:n_nodes],
                        adj[2 * P * j : 2 * P * (j + 1), :]
                        .rearrange("(t p) k -> p t k", p=P),
                    )
                else:
                    for s, i in enumerate(tiles):
                        h = mh[i]
                        nc.gpsimd.dma_start(
                            nat2[:h, s, 0:n_nodes],
                            adj[P * i : P * i + h, :],
                        )
                for s, i in enumerate(tiles):
                    h = mh[i]
                    # rowsum on ACT (fp32 accumulator), output discarded
                    nc.scalar.activation(
                        red_scr[:h, 0:n_nodes], nat2[:h, s, 0:n_nodes],
                        AF.Copy, accum_out=deg_col[:h, i : i + 1],
                    )
                    # A^T blocks via PE; drain 4 blocks per DVE copy
                    for kb0 in range(0, KT, KB_GRP):
                        kbn = min(KB_GRP, KT - kb0)
                        bank = bank_pool.tile([P, KB_GRP * P], at_dtype,
                                              tag="tb")
                        for kk in range(kbn):
                            c0 = s * KTP + (kb0 + kk) * P
                            nc.tensor.transpose(
                                bank[:P, kk * P : kk * P + P],
                                nat[:P, c0 : c0 + P],
                                identB[:P, :P],
                            )
                        nc.vector.tensor_copy(
                            AT4[:, i, kb0 : kb0 + kbn, :],
                            bank[:P, 0 : kbn * P],
                        )

                # x/h0 epilogue transposes ride along
                for i in tiles:
                    h = mh[i]
                    xt_ps = ptx_pool.tile([P, P], dt.float32, tag="sm")
                    nc.tensor.transpose(
                        xt_ps[:F, :h], xn_all[:h, i * F : i * F + F], ident[:h, :h]
                    )
                    nc.vector.tensor_copy(xT[:, P * i : P * i + h], xt_ps[:F, :h])
                    ht_ps = ptx_pool.tile([P, P], dt.float32, tag="sm")
                    nc.tensor.transpose(
                        ht_ps[:F, :h], hn_all[:h, i * F : i * F + F], ident[:h, :h]
                    )
                    nc.scalar.activation(
                        h0aT[:, P * i : P * i + h], ht_ps[:F, :h], AF.Copy,
                        scale=alpha,
                    )

            bank_ctx.__exit__(None, None, None)

            # ---------------- degree tail: dis = rsqrt(deg+1), exchange -------
            degp = deg_pool.tile([P, MT], dt.float32, tag="degp")
            nc.vector.tensor_scalar_add(degp[:], deg_col[:], 1.0)
            dis_col = deg_pool.tile([P, MT], dt.float32, tag="dis_col")
            nc.vector.reciprocal(dis_col[:], degp[:])
            nc.scalar.sqrt(dis_col[:], dis_col[:])

            degT_ps = ptx_pool.tile([P, P], dt.float32, tag="sm")
            nc.tensor.transpose(degT_ps[:MT, :P], dis_col[:P, :MT], ident[:P, :P])
            disT = deg_pool.tile([P, P], dt.float32, tag="disT")
            nc.vector.tensor_copy(disT[:MT, :P], degT_ps[:MT, :P])

            dis_loc_d = dram.tile([m_rows], dt.float32)
            dis_full_d = dram.tile([n_nodes], dt.float32)
            st_insts = []
            if mfull:
                st_insts.append(nc.gpsimd.dma_start(
                    dis_loc_d[0 : mfull * P].rearrange("(a b) -> a b", b=P),
                    disT[0:mfull, :],
                ))
            if mtail:
                st_insts.append(nc.gpsimd.dma_start(
                    dis_loc_d[mfull * P : m_rows].rearrange("(a b) -> a b", a=1),
                    disT[mfull : mfull + 1, 0:mtail],
                ))
            # local dis row reload happens BEFORE the collective on the
            # gpsimd FIFO so qT prep can overlap the AllGather
            vecs = tvec_pool.tile([P, m_rows], dt.float32)
            dis_row = vecs[0:1, :]
            ld_row = nc.gpsimd.dma_start(
                dis_row[0:1, 0:m_rows],
                dis_loc_d[:].rearrange("(a b) -> a b", a=1),
            )
            for st in st_insts:
                _adh(ld_row.ins, st.ins, sync=True,
                     reason="dis row reload after both dis stores")
            # Explicitly fence the collective's READ of dis_loc_d on both
            # stores' completion: the DRAM-tile dep does not reliably hold
            # the CC stream's read (v3 corrupted the last dis elements when
            # the AG fired right behind the tail store).
            ag = nc.gpsimd.collective_compute(
                "AllGather",
                mybir.AluOpType.bypass,
                replica_groups=groups,
                ins=[dis_loc_d[:]],
                outs=[dis_full_d[:]],
            )
            for st in st_insts:
                _adh(ag.ins, st.ins, sync=True,
                     reason="collective reads dis_loc_d after both stores")

            # broadcast dis across partitions via PE ones-matmuls (keeps the
            # gpsimd FIFO free so the AllGather dispatches immediately), then
            # rs = c1*dis, s1 = c1*dis^2, qT = s1*x^T + alpha*h0^T -- all
            # overlapping the collective.
            s1_b = tvec_pool.tile([P, m_rows], dt.float32, tag="s1_b")
            rs_b = tvec_pool.tile([P, m_rows], dt.float32, tag="rs_b")
            qT = xs_pool.tile([P, m_rows], dt.float32, tag="qT")
            with tc.tile_pool(name="bc_ps", bufs=2, space="PSUM") as bc_pool:
                for s in range(0, m_rows, CHUNK):
                    wc = min(CHUNK, m_rows - s)
                    bc = bc_pool.tile([P, CHUNK], dt.float32)
                    nc.tensor.matmul(
                        bc[:P, 0:wc], ones1[0:1, :P], dis_row[0:1, s : s + wc],
                        start=True, stop=True,
                    )
                    nc.vector.tensor_copy(s1_b[:, s : s + wc], bc[:P, 0:wc])
            nc.vector.tensor_scalar_mul(rs_b[:], s1_b[:], c1)
            nc.vector.tensor_mul(s1_b[:], s1_b[:], rs_b[:])
            nc.vector.tensor_mul(qT[:], xT[:], s1_b[:])
            nc.vector.tensor_add(qT[:], qT[:], h0aT[:])

            # PE warm-up matmuls spanning the collective so phase-2 matmuls
            # start at the warm 2.4 GHz clock (fp32 on thetaB, no deps)
            with tc.tile_pool(name="warm_ps", bufs=1, space="PSUM") as warm_pool:
                wp = warm_pool.tile([P, P], dt.float32)
                for j in range(N_WARM):
                    nc.tensor.matmul(
                        wp[:P, 0:P],
                        theta_sb[:F, :F],
                        theta_sb[:F, :F],
                        start=(j == 0),
                        stop=(j == N_WARM - 1),
                    )

            # global dis -> per k-block column layout [P, KT]
            if kfull:
                dg_ld = nc.gpsimd.dma_start(
                    dgT[0:kfull, 0:P],
                    dis_full_d[0 : kfull * P].rearrange("(a b) -> a b", b=P),
                )
                _adh(dg_ld.ins, ag.ins, sync=True,
                     reason="dis_full reload after collective completes")
            if ktail:
                dg_ld = nc.gpsimd.dma_start(
                    dgT[kfull : kfull + 1, 0:ktail],
                    dis_full_d[kfull * P : n_nodes],
                )
                _adh(dg_ld.ins, ag.ins, sync=True,
                     reason="dis_full reload after collective completes")
            dg_ps = ptx_pool.tile([P, P], dt.float32, tag="sm")
            nc.tensor.transpose(dg_ps[:P, :KT], dgT[:KT, :P], ident[:KT, :KT])
            disg = deg_pool.tile([P, KT], dt.float32, tag="disg")
            nc.vector.tensor_copy(disg[:], dg_ps[:P, :KT])

            if debug_dump:
                dbg_at = nc.dram_tensor(
                    "dbg_at", [P, MT * KTP], at_dtype, kind="ExternalOutput"
                )
                nc.sync.dma_start(dbg_at[:], AT[:])
                dbg_deg = nc.dram_tensor(
                    "dbg_deg", [P, MT], dt.float32, kind="ExternalOutput"
                )
                nc.sync.dma_start(dbg_deg[:], deg_col[:])
                dbg_disg = nc.dram_tensor(
                    "dbg_disg", [P, P], dt.float32, kind="ExternalOutput"
                )
                nc.sync.dma_start(dbg_disg[:], dgT[:])

            # ---------------- Phase 2: xs scaling + matmuls + epilogue --------
            # xs = D^-1/2 x in bf16; the scale ops stream on DVE just ahead
            # of the chunk matmuls on PE.
            xs = xs_pool.tile([P, KT * F], at_dtype, tag="xs")
            for kb in range(KT):
                w = kw[kb]
                nc.vector.tensor_scalar_mul(
                    xs[:w, kb * F : kb * F + F],
                    xg[:w, kb * F : kb * F + F],
                    disg[:w, kb : kb + 1],
                )

            out_sb = xs_pool.tile([P, MT * F], dt.float32, tag="out_sb")
            with tc.tile_pool(name="hi_ps", bufs=2, space="PSUM") as hi_pool, \
                 tc.tile_pool(name="o2_ps", bufs=2, space="PSUM") as o2_pool:
                hi_tiles = []

                def emit_mms(ci):
                    s, wc, ia, ib, tw = mchunks[ci]
                    hiT = hi_pool.tile([P, CHUNK], dt.float32)
                    for kb in range(KT):
                        w = kw[kb]
                        nc.tensor.matmul(
                            hiT[:F, 0:wc],
                            xs[:w, kb * F : kb * F + F],
                            AT4[:w, ia:ib, kb, 0:tw],
                            start=(kb == 0),
                            stop=(kb == KT - 1),
                        )
                    hi_tiles.append(hiT)

                def emit_epilogue(ci):
                    s, wc, ia, ib, tw = mchunks[ci]
                    hiT = hi_tiles[ci]
                    supT = sup_pool.tile([P, CHUNK], dt.float32)
                    nc.vector.tensor_mul(
                        supT[:, 0:wc], hiT[:F, 0:wc], rs_b[:, s : s + wc]
                    )
                    nc.vector.tensor_add(
                        supT[:, 0:wc], supT[:, 0:wc], qT[:, s : s + wc]
                    )
                    o2T = o2_pool.tile([P, CHUNK], dt.float32)
                    nc.tensor.matmul(
                        o2T[:F, 0:wc], thetaB[:F, :F], supT[:F, 0:wc],
                        start=True, stop=True,
                    )
                    outT = outc_pool.tile([P, CHUNK], dt.float32)
                    nc.vector.scalar_tensor_tensor(
                        outT[:, 0:wc], supT[:, 0:wc], 1.0 - beta, o2T[:F, 0:wc],
                        mybir.AluOpType.mult, mybir.AluOpType.add,
                    )
                    # back to natural [m, f] into the staging tile
                    for off in range(0, wc, P):
                        hh = min(P, wc - off)
                        ti_ = ia + off // P
                        ot_ps = ptx_pool.tile([P, P], dt.float32, tag="sm")
                        nc.tensor.transpose(
                            ot_ps[:hh, :F], outT[:F, off : off + hh], ident[:F, :F]
                        )
                        nc.vector.tensor_copy(
                            out_sb[:hh, ti_ * F : ti_ * F + F], ot_ps[:hh, :F]
                        )
                    # store this chunk's rows
                    if tw == P:
                        nc.gpsimd.dma_start(
                            out_d[s : s + wc, :].rearrange("(i p) f -> p i f", p=P),
                            out_sb[:].rearrange("p (i f) -> p i f", i=MT)[:, ia:ib, :],
                        )
                    else:
                        nc.gpsimd.dma_start(
                            out_d[s : s + wc, :], out_sb[0:wc, ia * F : ia * F + F]
                        )

                emit_mms(0)
                for ci in range(1, len(mchunks)):
                    emit_mms(ci)
                    emit_epilogue(ci - 1)
                emit_epilogue(len(mchunks) - 1)

    nc.compile()
    return nc


def make_in_maps(x, adj, h0, theta, n_cores):
    m = x.shape[1] // 2
    in_maps = []
    for c in range(n_cores):
        b, half = c // 2, c % 2
        r0 = half * m
        in_maps.append(
            {
                "adj_rows": adj[b, r0 : r0 + m, :],
                "x_full": x[b],
                "x_loc": x[b, r0 : r0 + m, :],
                "h0_loc": h0[b, r0 : r0 + m, :],
                "theta": theta,
            }
        )
    return in_maps


_CACHE = {}


def _get_program(key, *args, **kwargs):
    if key not in _CACHE:
        _CACHE[key] = build_program(*args, **kwargs)
    return _CACHE[key]


def kernel(x, adj, h0, theta, lamda, alpha, l):
    x = np.asarray(x, dtype=np.float32)
    adj = np.asarray(adj, dtype=np.float32)
    h0 = np.asarray(h0, dtype=np.float32)
    theta = np.asarray(theta, dtype=np.float32)
    lamda_f = float(np.asarray(lamda))
    alpha_f = float(np.asarray(alpha))
    l_f = float(np.asarray(l))
    beta_f = float(math.log(lamda_f / l_f + 1.0))

    B, N, Fdim = x.shape
    assert (B, N, Fdim) == (B_FULL, N_FULL, F)
    M = N // 2

    nc = _get_program(
        ("full", alpha_f, beta_f), N, M, N_CORES_FULL, alpha_f, beta_f
    )

    in_maps = make_in_maps(x, adj, h0, theta, N_CORES_FULL)
    res = bass_utils.run_bass_kernel_spmd(
        nc, in_maps, list(range(N_CORES_FULL))
    ).results

    out = np.empty((B, N, Fdim), dtype=np.float32)
    for c in range(N_CORES_FULL):
        b, half = c // 2, c % 2
        out[b, half * M : (half + 1) * M, :] = res[c]["out"]
    return out


# revision 22
# speedup vs baseline: 1.7493x; 1.0784x over previous
"""GCN2Conv (variant=False) Trainium2 kernel — v3.

out = beta * (support @ theta) + (1-beta) * support
support = (1-alpha) * (D^-1/2 (A+I) D^-1/2 @ x) + alpha * h0
beta = log(lamda/l + 1)

Sharding: B=4 graphs over 8 cores -> 2 cores per graph, each owning
m_rows = N/2 = 1500 adjacency rows (global column order everywhere).

Key lessons baked in (from v1/v2 traces):
  - Tile recycles 8 DMA-completion semaphore lanes round-robin PER DGE
    CLASS, and engine queues are strict FIFO: a sem-wait parked in front
    of a DMA stalls everything behind it. So the SWDGE queue carries ONLY
    the 12 adj cast-loads (back-to-back, no interleaved small DMAs), all
    other loads ride the two HWDGE rings, and the stream ring is 6 tiles
    deep so the WAR fence (load i vs xbar transpose i-6) never binds.
  - fp32->bf16 cast happens inside the SWDGE DMA (wire-rate ~26 GB/s per
    engine x16, measured) -- no ACT cast pass at all.
  - xbar transposes alternate between the sync and scalar HWDGE rings so
    two drain concurrently; collectives are fenced after the last two.
  - The degree tail runs in [128, MT] column layout; the row-broadcast of
    dis uses PE ones-matmuls (NOT gpsimd partition_broadcast, which would
    park behind the AllGather in the gpsimd FIFO) so qT/rs_b/s1_b prep
    overlaps the collective.
  - Phase-2: xs scaling (DVE) is emitted interleaved ahead of the chunk
    matmuls so PE chases the scale stream; epilogue is software-pipelined
    one chunk behind the matmuls.
"""

import math
import sys

import numpy as np

sys.path.insert(0, "/opt/trn_rl_repo")

import concourse.bacc as bacc
import concourse.mybir as mybir
import concourse.tile as tile
from concourse import bass_utils, masks
from concourse.mybir import dt

AF = mybir.ActivationFunctionType

F = 128          # feature dim (= theta size), fixed
P = 128          # SBUF partitions
CHUNK = 512      # phase-2 m-chunk width (one fp32 PSUM bank)

B_FULL, N_FULL = 4, 3000
N_CORES_FULL = 8
M_FULL = N_FULL // 2

RING_PAIRS = 3   # adj bf16 stream pair-tiles in flight
N_WARM = 24      # PE warm-up matmuls spanning the collective


def _tile_sizes(total, step):
    return [min(step, total - s) for s in range(0, total, step)]


def build_program(n_nodes, m_rows, n_cores, alpha, beta, at_dtype=dt.bfloat16,
                  debug_dump=False):
    """Build the SPMD Bass program (identical on every core).

    Per-core external inputs (host pre-slices):
      adj_rows [m_rows, n_nodes], x_full [n_nodes, F], x_loc [m_rows, F],
      h0_loc [m_rows, F], theta [F, F].
    Output: out [m_rows, F].
    Cores 2g, 2g+1 own rows [0:m_rows], [m_rows:2*m_rows] of graph g.
    """
    assert n_nodes == 2 * m_rows
    c1 = 1.0 - alpha

    KT = math.ceil(n_nodes / P)        # k blocks (adj cols / nodes)
    kw = _tile_sizes(n_nodes, P)
    MT = math.ceil(m_rows / P)         # local m tiles
    mh = _tile_sizes(m_rows, P)
    mfull, mtail = m_rows // P, m_rows % P
    kfull, ktail = n_nodes // P, n_nodes % P
    KTP = KT * P
    # phase-2 chunks: groups of up to 4 full m-tiles (512 cols) or the tail
    # tile alone -- each chunk is one contiguous piece of A^T and gets its
    # own PSUM accumulation bank
    mchunks = []
    ti = 0
    while ti < MT:
        if mh[ti] == P:
            tj = ti
            while tj < MT and mh[tj] == P and tj - ti < 4:
                tj += 1
            mchunks.append((ti * P, (tj - ti) * P, ti, tj, P))
            ti = tj
        else:
            mchunks.append((ti * P, mh[ti], ti, ti + 1, mh[ti]))
            ti += 1

    nc = bacc.Bacc(
        "TRN2", target_bir_lowering=False, debug=False, num_devices=n_cores
    )
    adj = nc.dram_tensor("adj_rows", [m_rows, n_nodes], dt.float32, kind="ExternalInput")
    x_full = nc.dram_tensor("x_full", [n_nodes, F], dt.float32, kind="ExternalInput")
    x_loc = nc.dram_tensor("x_loc", [m_rows, F], dt.float32, kind="ExternalInput")
    h0_loc = nc.dram_tensor("h0_loc", [m_rows, F], dt.float32, kind="ExternalInput")
    theta = nc.dram_tensor("theta", [F, F], dt.float32, kind="ExternalInput")
    out_d = nc.dram_tensor("out", [m_rows, F], dt.float32, kind="ExternalOutput")

    groups = [[2 * g, 2 * g + 1] for g in range(n_cores // 2)]

    with tile.TileContext(nc) as tc:
        from contextlib import ExitStack
        from concourse.tile import add_dep_helper as _adh

        with ExitStack() as ctx:
            ep = ctx.enter_context

            consts = ep(tc.tile_pool(name="consts", bufs=1))
            at_pool = ep(tc.tile_pool(name="at", bufs=1))
            stream_pool = ep(tc.tile_pool(name="stream", bufs=RING_PAIRS))
            deg_pool = ep(tc.tile_pool(name="deg", bufs=1))
            xs_pool = ep(tc.tile_pool(name="xs", bufs=1))
            tvec_pool = ep(tc.tile_pool(name="tvec", bufs=1))
            sup_pool = ep(tc.tile_pool(name="sup", bufs=2))
            outc_pool = ep(tc.tile_pool(name="outc", bufs=2))
            ptx_pool = ep(tc.tile_pool(name="ptx", bufs=2, space="PSUM"))
            dram = ep(tc.tile_pool(name="dram", bufs=1, space="DRAM"))

            ident = consts.tile([P, P], dt.float32)
            masks.make_identity(nc, ident[:])
            identB = consts.tile([P, P], at_dtype)
            masks.make_identity(nc, identB[:])

            ones1 = consts.tile([1, P], dt.float32)
            nc.gpsimd.memset(ones1[:], 1.0)

            # local degree columns: col i = row sums of local m-tile i
            deg_col = deg_pool.tile([P, MT], dt.float32)
            nc.gpsimd.memset(deg_col[:], 0.0)  # garbage lanes stay rsqrt-safe
            dgT = deg_pool.tile([P, P], dt.float32, tag="dgT")
            nc.gpsimd.memset(dgT[:KT, :], 1.0)

            # theta is a single-descriptor HWDGE load; everything else with
            # rearranged (many-small-descriptor) patterns goes on SWDGE --
            # HWDGE descriptor generation for those blocks its issuing
            # engine's queue for ~18us (measured), which starved the ACT
            # rowsums in v5.
            theta_sb = consts.tile([F, F], dt.float32)
            nc.sync.dma_start(theta_sb[:], theta[:])
            thetaB = consts.tile([F, F], dt.float32)
            nc.vector.tensor_scalar_mul(thetaB[:], theta_sb[:], beta)

            xg = xs_pool.tile([P, KT * F], dt.float32)
            xn_all = xs_pool.tile([P, MT * F], dt.float32, tag="xn_all")
            hn_all = xs_pool.tile([P, MT * F], dt.float32, tag="hn_all")

            def emit_xghn_loads():
                # SWDGE, emitted AFTER the adj pair-loads so they queue
                # behind the critical stream
                if kfull:
                    nc.gpsimd.dma_start(
                        xg[:].rearrange("p (kb f) -> p kb f", kb=KT)[:, 0:kfull, :],
                        x_full[0 : kfull * P, :].rearrange("(kb p) f -> p kb f", p=P),
                    )
                if ktail:
                    nc.gpsimd.dma_start(
                        xg[0:ktail, kfull * F : (kfull + 1) * F],
                        x_full[kfull * P : n_nodes, :],
                    )
                for src, dst in ((x_loc, xn_all), (h0_loc, hn_all)):
                    if mfull:
                        nc.gpsimd.dma_start(
                            dst[:].rearrange("p (i f) -> p i f", i=MT)[:, 0:mfull, :],
                            src[0 : mfull * P, :].rearrange("(i p) f -> p i f", p=P),
                        )
                    if mtail:
                        nc.gpsimd.dma_start(
                            dst[0:mtail, mfull * F : (mfull + 1) * F],
                            src[mfull * P : m_rows, :],
                        )

            # A^T resident in SBUF: [k_local, (m_tile, kb, m_local)] -- one
            # contiguous [KT, 128] region per m-tile so the blocked xbar
            # transpose writes it in a single instruction
            AT = at_pool.tile([P, MT * KTP], at_dtype)
            AT4 = AT[:].rearrange("p (i kb m) -> p i kb m", i=MT, kb=KT)

            xT = xs_pool.tile([P, m_rows], dt.float32, tag="xT")
            h0aT = xs_pool.tile([P, m_rows], dt.float32, tag="h0aT")

            # ---------------- Phase 1: stream adj, rowsum + transpose ----------
            # PAIR granularity loads: one SWDGE cast-load (fp32->bf16) covers
            # two m-tiles.  NO xbar: the DMA-transpose corrupts data when two
            # of them overlap on the two HWDGE rings, and Tile serializes
            # every SWDGE DMA against in-flight xbars anyway.  Instead the
            # (otherwise idle) PE transposes A in [128,128] blocks, 4 blocks
            # into one PSUM bank, and DVE drains each bank with a single
            # 512-wide copy into bf16 A^T.  ACT does the rowsums (Copy with
            # accum_out into a throwaway scratch so the PE transposes never
            # serialize behind it).  Every access is visible to Tile's dep
            # tracker -- no manual fences anywhere in the pipeline.
            pairs = [(j, list(range(2 * j, min(2 * j + 2, MT))))
                     for j in range((MT + 1) // 2)]
            KB_GRP = 4              # transposed blocks per PSUM bank drain
            red_scr = xs_pool.tile([P, KTP], at_dtype, tag="red_scr")
            bank_ctx = tc.tile_pool(name="bank_ps", bufs=4, space="PSUM")
            bank_pool = bank_ctx.__enter__()

            # Early PE warm-up so phase-1 transposes start at 2.4 GHz
            # (overlaps the first loads; PE is idle anyway)
            with tc.tile_pool(name="warm0_ps", bufs=1, space="PSUM") as w0_pool:
                wp0 = w0_pool.tile([P, P], dt.float32)
                for j in range(N_WARM):
                    nc.tensor.matmul(
                        wp0[:P, 0:P], theta_sb[:F, :F], theta_sb[:F, :F],
                        start=(j == 0), stop=(j == N_WARM - 1),
                    )

            nat_slots = {}

            def emit_pair_load(j, tiles):
                nat = stream_pool.tile([P, 2 * KTP], at_dtype, tag="nat")
                nat_slots[j] = nat
                nat2 = nat[:].rearrange("p (t k) -> p t k", t=2)
                if len(tiles) == 2 and mh[tiles[0]] == P and mh[tiles[1]] == P:
                    nc.gpsimd.dma_start(
                        nat2[:, :, 0:n_nodes],
                        adj[2 * P * j : 2 * P * (j + 1), :]
                        .rearrange("(t p) k -> p t k", p=P),
                    )
                else:
                    for s, i in enumerate(tiles):
                        h = mh[i]
                        nc.gpsimd.dma_start(
                            nat2[:h, s, 0:n_nodes],
                            adj[P * i : P * i + h, :],
                        )

            def emit_pair_reduces(j, tiles):
                nat2 = nat_slots[j][:].rearrange("p (t k) -> p t k", t=2)
                for s, i in enumerate(tiles):
                    h = mh[i]
                    # rowsum on ACT (fp32 accumulator), output discarded
                    nc.scalar.activation(
                        red_scr[:h, 0:n_nodes], nat2[:h, s, 0:n_nodes],
                        AF.Copy, accum_out=deg_col[:h, i : i + 1],
                    )

            def emit_pair_at(j, tiles):
                # A^T blocks via PE; drain 4 blocks per DVE copy
                nat = nat_slots[j]
                for s, i in enumerate(tiles):
                    for kb0 in range(0, KT, KB_GRP):
                        kbn = min(KB_GRP, KT - kb0)
                        bank = bank_pool.tile([P, KB_GRP * P], at_dtype,
                                              tag="tb")
                        for kk in range(kbn):
                            c0 = s * KTP + (kb0 + kk) * P
                            nc.tensor.transpose(
                                bank[:P, kk * P : kk * P + P],
                                nat[:P, c0 : c0 + P],
                                identB[:P, :P],
                            )
                        nc.vector.tensor_copy(
                            AT4[:, i, kb0 : kb0 + kbn, :],
                            bank[:P, 0 : kbn * P],
                        )

            for j, tiles in pairs[:-1]:
                emit_pair_load(j, tiles)
                emit_pair_reduces(j, tiles)
                emit_pair_at(j, tiles)
            jL, tilesL = pairs[-1]
            emit_pair_load(jL, tilesL)
            emit_xghn_loads()
            emit_pair_reduces(jL, tilesL)
            # deg tail is emitted BEFORE the last pair's A^T transposes so
            # the dis stores + AllGather dispatch ~8us earlier; the last A^T
            # blocks transpose during the collective.

            # ---------------- degree tail: dis = rsqrt(deg+1), exchange -------
            degp = deg_pool.tile([P, MT], dt.float32, tag="degp")
            nc.vector.tensor_scalar_add(degp[:], deg_col[:], 1.0)
            dis_col = deg_pool.tile([P, MT], dt.float32, tag="dis_col")
            nc.vector.reciprocal(dis_col[:], degp[:])
            nc.scalar.sqrt(dis_col[:], dis_col[:])

            degT_ps = ptx_pool.tile([P, P], dt.float32, tag="sm")
            nc.tensor.transpose(degT_ps[:MT, :P], dis_col[:P, :MT], ident[:P, :P])
            disT = deg_pool.tile([P, P], dt.float32, tag="disT")
            nc.vector.tensor_copy(disT[:MT, :P], degT_ps[:MT, :P])

            dis_loc_d = dram.tile([m_rows], dt.float32)
            dis_full_d = dram.tile([n_nodes], dt.float32)
            st_insts = []
            if mfull:
                st_insts.append(nc.gpsimd.dma_start(
                    dis_loc_d[0 : mfull * P].rearrange("(a b) -> a b", b=P),
                    disT[0:mfull, :],
                ))
            if mtail:
                st_insts.append(nc.gpsimd.dma_start(
                    dis_loc_d[mfull * P : m_rows].rearrange("(a b) -> a b", a=1),
                    disT[mfull : mfull + 1, 0:mtail],
                ))
            # local dis row reload happens BEFORE the collective on the
            # gpsimd FIFO so qT prep can overlap the AllGather
            vecs = tvec_pool.tile([P, m_rows], dt.float32)
            dis_row = vecs[0:1, :]
            ld_row = nc.gpsimd.dma_start(
                dis_row[0:1, 0:m_rows],
                dis_loc_d[:].rearrange("(a b) -> a b", a=1),
            )
            for st in st_insts:
                _adh(ld_row.ins, st.ins, sync=True,
                     reason="dis row reload after both dis stores")
            # Explicitly fence the collective's READ of dis_loc_d on both
            # stores' completion: the DRAM-tile dep does not reliably hold
            # the CC stream's read (v3 corrupted the last dis elements when
            # the AG fired right behind the tail store).
            ag = nc.gpsimd.collective_compute(
                "AllGather",
                mybir.AluOpType.bypass,
                replica_groups=groups,
                ins=[dis_loc_d[:]],
                outs=[dis_full_d[:]],
            )
            for st in st_insts:
                _adh(ag.ins, st.ins, sync=True,
                     reason="collective reads dis_loc_d after both stores")

            # last pair's A^T transposes + the x/h0 epilogue transposes run
            # during the collective
            emit_pair_at(jL, tilesL)
            for i in range(MT):
                h = mh[i]
                xt_ps = ptx_pool.tile([P, P], dt.float32, tag="sm")
                nc.tensor.transpose(
                    xt_ps[:F, :h], xn_all[:h, i * F : i * F + F], ident[:h, :h]
                )
                nc.vector.tensor_copy(xT[:, P * i : P * i + h], xt_ps[:F, :h])
                ht_ps = ptx_pool.tile([P, P], dt.float32, tag="sm")
                nc.tensor.transpose(
                    ht_ps[:F, :h], hn_all[:h, i * F : i * F + F], ident[:h, :h]
                )
                nc.scalar.activation(
                    h0aT[:, P * i : P * i + h], ht_ps[:F, :h], AF.Copy,
                    scale=alpha,
                )
            bank_ctx.__exit__(None, None, None)

            # broadcast dis across partitions via PE ones-matmuls (keeps the
            # gpsimd FIFO free so the AllGather dispatches immediately), then
            # rs = c1*dis, s1 = c1*dis^2, qT = s1*x^T + alpha*h0^T -- all
            # overlapping the collective.
            s1_b = tvec_pool.tile([P, m_rows], dt.float32, tag="s1_b")
            rs_b = tvec_pool.tile([P, m_rows], dt.float32, tag="rs_b")
            qT = xs_pool.tile([P, m_rows], dt.float32, tag="qT")
            with tc.tile_pool(name="bc_ps", bufs=2, space="PSUM") as bc_pool:
                for s in range(0, m_rows, CHUNK):
                    wc = min(CHUNK, m_rows - s)
                    bc = bc_pool.tile([P, CHUNK], dt.float32)
                    nc.tensor.matmul(
                        bc[:P, 0:wc], ones1[0:1, :P], dis_row[0:1, s : s + wc],
                        start=True, stop=True,
                    )
                    nc.vector.tensor_copy(s1_b[:, s : s + wc], bc[:P, 0:wc])
            nc.vector.tensor_scalar_mul(rs_b[:], s1_b[:], c1)
            nc.vector.tensor_mul(s1_b[:], s1_b[:], rs_b[:])
            nc.vector.tensor_mul(qT[:], xT[:], s1_b[:])
            nc.vector.tensor_add(qT[:], qT[:], h0aT[:])

            # PE warm-up matmuls spanning the collective so phase-2 matmuls
            # start at the warm 2.4 GHz clock (fp32 on thetaB, no deps)
            with tc.tile_pool(name="warm_ps", bufs=1, space="PSUM") as warm_pool:
                wp = warm_pool.tile([P, P], dt.float32)
                for j in range(N_WARM):
                    nc.tensor.matmul(
                        wp[:P, 0:P],
                        theta_sb[:F, :F],
                        theta_sb[:F, :F],
                        start=(j == 0),
                        stop=(j == N_WARM - 1),
                    )

            # global dis -> per k-block column layout [P, KT]
            if kfull:
                dg_ld = nc.gpsimd.dma_start(
                    dgT[0:kfull, 0:P],
                    dis_full_d[0 : kfull * P].rearrange("(a b) -> a b", b=P),
                )
                _adh(dg_ld.ins, ag.ins, sync=True,
                     reason="dis_full reload after collective completes")
            if ktail:
                dg_ld = nc.gpsimd.dma_start(
                    dgT[kfull : kfull + 1, 0:ktail],
                    dis_full_d[kfull * P : n_nodes],
                )
                _adh(dg_ld.ins, ag.ins, sync=True,
                     reason="dis_full reload after collective completes")
            dg_ps = ptx_pool.tile([P, P], dt.float32, tag="sm")
            nc.tensor.transpose(dg_ps[:P, :KT], dgT[:KT, :P], ident[:KT, :KT])
            disg = deg_pool.tile([P, KT], dt.float32, tag="disg")
            nc.vector.tensor_copy(disg[:], dg_ps[:P, :KT])

            if debug_dump:
                dbg_at = nc.dram_tensor(
                    "dbg_at", [P, MT * KTP], at_dtype, kind="ExternalOutput"
                )
                nc.sync.dma_start(dbg_at[:], AT[:])
                dbg_deg = nc.dram_tensor(
                    "dbg_deg", [P, MT], dt.float32, kind="ExternalOutput"
                )
                nc.sync.dma_start(dbg_deg[:], deg_col[:])
                dbg_disg = nc.dram_tensor(
                    "dbg_disg", [P, P], dt.float32, kind="ExternalOutput"
                )
                nc.sync.dma_start(dbg_disg[:], dgT[:])

            # ---------------- Phase 2: xs scaling + matmuls + epilogue --------
            # xs = D^-1/2 x in bf16; the scale ops stream on DVE just ahead
            # of the chunk matmuls on PE.
            xs = xs_pool.tile([P, KT * F], at_dtype, tag="xs")
            for kb in range(KT):
                w = kw[kb]
                nc.vector.tensor_scalar_mul(
                    xs[:w, kb * F : kb * F + F],
                    xg[:w, kb * F : kb * F + F],
                    disg[:w, kb : kb + 1],
                )

            out_sb = xs_pool.tile([P, MT * F], dt.float32, tag="out_sb")
            with tc.tile_pool(name="hi_ps", bufs=2, space="PSUM") as hi_pool, \
                 tc.tile_pool(name="o2_ps", bufs=2, space="PSUM") as o2_pool:
                hi_tiles = []

                def emit_mms(ci):
                    s, wc, ia, ib, tw = mchunks[ci]
                    hiT = hi_pool.tile([P, CHUNK], dt.float32)
                    for kb in range(KT):
                        w = kw[kb]
                        nc.tensor.matmul(
                            hiT[:F, 0:wc],
                            xs[:w, kb * F : kb * F + F],
                            AT4[:w, ia:ib, kb, 0:tw],
                            start=(kb == 0),
                            stop=(kb == KT - 1),
                        )
                    hi_tiles.append(hiT)

                def emit_epilogue(ci):
                    s, wc, ia, ib, tw = mchunks[ci]
                    hiT = hi_tiles[ci]
                    supT = sup_pool.tile([P, CHUNK], dt.float32)
                    nc.vector.tensor_mul(
                        supT[:, 0:wc], hiT[:F, 0:wc], rs_b[:, s : s + wc]
                    )
                    nc.vector.tensor_add(
                        supT[:, 0:wc], supT[:, 0:wc], qT[:, s : s + wc]
                    )
                    o2T = o2_pool.tile([P, CHUNK], dt.float32)
                    nc.tensor.matmul(
                        o2T[:F, 0:wc], thetaB[:F, :F], supT[:F, 0:wc],
                        start=True, stop=True,
                    )
                    outT = outc_pool.tile([P, CHUNK], dt.float32)
                    nc.vector.scalar_tensor_tensor(
                        outT[:, 0:wc], supT[:, 0:wc], 1.0 - beta, o2T[:F, 0:wc],
                        mybir.AluOpType.mult, mybir.AluOpType.add,
                    )
                    # back to natural [m, f] into the staging tile
                    for off in range(0, wc, P):
                        hh = min(P, wc - off)
                        ti_ = ia + off // P
                        ot_ps = ptx_pool.tile([P, P], dt.float32, tag="sm")
                        nc.tensor.transpose(
                            ot_ps[:hh, :F], outT[:F, off : off + hh], ident[:F, :F]
                        )
                        nc.vector.tensor_copy(
                            out_sb[:hh, ti_ * F : ti_ * F + F], ot_ps[:hh, :F]
                        )
                    # store this chunk's rows
                    if tw == P:
                        nc.gpsimd.dma_start(
                            out_d[s : s + wc, :].rearrange("(i p) f -> p i f", p=P),
                            out_sb[:].rearrange("p (i f) -> p i f", i=MT)[:, ia:ib, :],
                        )
                    else:
                        nc.gpsimd.dma_start(
                            out_d[s : s + wc, :], out_sb[0:wc, ia * F : ia * F + F]
                        )

                emit_mms(0)
                for ci in range(1, len(mchunks)):
                    emit_mms(ci)
                    emit_epilogue(ci - 1)
                emit_epilogue(len(mchunks) - 1)

    nc.compile()
    return nc


def make_in_maps(x, adj, h0, theta, n_cores):
    m = x.shape[1] // 2
    in_maps = []
    for c in range(n_cores):
        b, half = c // 2, c % 2
        r0 = half * m
        in_maps.append(
            {
                "adj_rows": adj[b, r0 : r0 + m, :],
                "x_full": x[b],
                "x_loc": x[b, r0 : r0 + m, :],
                "h0_loc": h0[b, r0 : r0 + m, :],
                "theta": theta,
            }
        )
    return in_maps


_CACHE = {}


def _get_program(key, *args, **kwargs):
    if key not in _CACHE:
        _CACHE[key] = build_program(*args, **kwargs)
    return _CACHE[key]


def kernel(x, adj, h0, theta, lamda, alpha, l):
    x = np.asarray(x, dtype=np.float32)
    adj = np.asarray(adj, dtype=np.float32)
    h0 = np.asarray(h0, dtype=np.float32)
    theta = np.asarray(theta, dtype=np.float32)
    lamda_f = float(np.asarray(lamda))
    alpha_f = float(np.asarray(alpha))
    l_f = float(np.asarray(l))
    beta_f = float(math.log(lamda_f / l_f + 1.0))

    B, N, Fdim = x.shape
    assert (B, N, Fdim) == (B_FULL, N_FULL, F)
    M = N // 2

    nc = _get_program(
        ("full", alpha_f, beta_f), N, M, N_CORES_FULL, alpha_f, beta_f
    )

    in_maps = make_in_maps(x, adj, h0, theta, N_CORES_FULL)
    res = bass_utils.run_bass_kernel_spmd(
        nc, in_maps, list(range(N_CORES_FULL))
    ).results

    out = np.empty((B, N, Fdim), dtype=np.float32)
    for c in range(N_CORES_FULL):
        b, half = c // 2, c % 2
        out[b, half * M : (half + 1) * M, :] = res[c]["out"]
    return out


# revision 26
# speedup vs baseline: 1.9723x; 1.1275x over previous
"""GCN2Conv (variant=False) Trainium2 kernel — v3.

out = beta * (support @ theta) + (1-beta) * support
support = (1-alpha) * (D^-1/2 (A+I) D^-1/2 @ x) + alpha * h0
beta = log(lamda/l + 1)

Sharding: B=4 graphs over 8 cores -> 2 cores per graph, each owning
m_rows = N/2 = 1500 adjacency rows (global column order everywhere).

Key lessons baked in (from v1/v2 traces):
  - Tile recycles 8 DMA-completion semaphore lanes round-robin PER DGE
    CLASS, and engine queues are strict FIFO: a sem-wait parked in front
    of a DMA stalls everything behind it. So the SWDGE queue carries ONLY
    the 12 adj cast-loads (back-to-back, no interleaved small DMAs), all
    other loads ride the two HWDGE rings, and the stream ring is 6 tiles
    deep so the WAR fence (load i vs xbar transpose i-6) never binds.
  - fp32->bf16 cast happens inside the SWDGE DMA (wire-rate ~26 GB/s per
    engine x16, measured) -- no ACT cast pass at all.
  - xbar transposes alternate between the sync and scalar HWDGE rings so
    two drain concurrently; collectives are fenced after the last two.
  - The degree tail runs in [128, MT] column layout; the row-broadcast of
    dis uses PE ones-matmuls (NOT gpsimd partition_broadcast, which would
    park behind the AllGather in the gpsimd FIFO) so qT/rs_b/s1_b prep
    overlaps the collective.
  - Phase-2: xs scaling (DVE) is emitted interleaved ahead of the chunk
    matmuls so PE chases the scale stream; epilogue is software-pipelined
    one chunk behind the matmuls.
"""

import math
import sys

import numpy as np

sys.path.insert(0, "/opt/trn_rl_repo")

import concourse.bacc as bacc
import concourse.mybir as mybir
import concourse.tile as tile
from concourse import bass_utils, masks
from concourse.mybir import dt

AF = mybir.ActivationFunctionType

F = 128          # feature dim (= theta size), fixed
P = 128          # SBUF partitions
CHUNK = 512      # phase-2 m-chunk width (one fp32 PSUM bank)

B_FULL, N_FULL = 4, 3000
N_CORES_FULL = 8
M_FULL = N_FULL // 2

RING_PAIRS = 4   # adj bf16 stream pair-tiles in flight
N_WARM = 24      # PE warm-up matmuls spanning the collective


def _tile_sizes(total, step):
    return [min(step, total - s) for s in range(0, total, step)]


def build_program(n_nodes, m_rows, n_cores, alpha, beta, at_dtype=dt.bfloat16,
                  debug_dump=False):
    """Build the SPMD Bass program (identical on every core).

    Per-core external inputs (host pre-slices):
      adj_rows [m_rows, n_nodes], x_full [n_nodes, F], x_loc [m_rows, F],
      h0_loc [m_rows, F], theta [F, F].
    Output: out [m_rows, F].
    Cores 2g, 2g+1 own rows [0:m_rows], [m_rows:2*m_rows] of graph g.
    """
    assert n_nodes == 2 * m_rows
    c1 = 1.0 - alpha

    KT = math.ceil(n_nodes / P)        # k blocks (adj cols / nodes)
    kw = _tile_sizes(n_nodes, P)
    MT = math.ceil(m_rows / P)         # local m tiles
    mh = _tile_sizes(m_rows, P)
    mfull, mtail = m_rows // P, m_rows % P
    kfull, ktail = n_nodes // P, n_nodes % P
    KTP = KT * P
    # phase-2 chunks: groups of up to 4 full m-tiles (512 cols) or the tail
    # tile alone -- each chunk is one contiguous piece of A^T and gets its
    # own PSUM accumulation bank
    mchunks = []
    ti = 0
    while ti < MT:
        if mh[ti] == P:
            tj = ti
            while tj < MT and mh[tj] == P and tj - ti < 4:
                tj += 1
            mchunks.append((ti * P, (tj - ti) * P, ti, tj, P))
            ti = tj
        else:
            mchunks.append((ti * P, mh[ti], ti, ti + 1, mh[ti]))
            ti += 1

    nc = bacc.Bacc(
        "TRN2", target_bir_lowering=False, debug=False, num_devices=n_cores
    )
    adj = nc.dram_tensor("adj_rows", [m_rows, n_nodes], dt.float32, kind="ExternalInput")
    x_full = nc.dram_tensor("x_full", [n_nodes, F], dt.float32, kind="ExternalInput")
    x_loc = nc.dram_tensor("x_loc", [m_rows, F], dt.float32, kind="ExternalInput")
    h0_loc = nc.dram_tensor("h0_loc", [m_rows, F], dt.float32, kind="ExternalInput")
    theta = nc.dram_tensor("theta", [F, F], dt.float32, kind="ExternalInput")
    out_d = nc.dram_tensor("out", [m_rows, F], dt.float32, kind="ExternalOutput")

    groups = [[2 * g, 2 * g + 1] for g in range(n_cores // 2)]

    with tile.TileContext(nc) as tc:
        from contextlib import ExitStack
        from concourse.tile import add_dep_helper as _adh

        with ExitStack() as ctx:
            ep = ctx.enter_context

            consts = ep(tc.tile_pool(name="consts", bufs=1))
            at_pool = ep(tc.tile_pool(name="at", bufs=1))
            stream_pool = ep(tc.tile_pool(name="stream", bufs=RING_PAIRS))
            deg_pool = ep(tc.tile_pool(name="deg", bufs=1))
            xs_pool = ep(tc.tile_pool(name="xs", bufs=1))
            tvec_pool = ep(tc.tile_pool(name="tvec", bufs=1))
            sup_pool = ep(tc.tile_pool(name="sup", bufs=2))
            outc_pool = ep(tc.tile_pool(name="outc", bufs=2))
            ptx_pool = ep(tc.tile_pool(name="ptx", bufs=2, space="PSUM"))
            dram = ep(tc.tile_pool(name="dram", bufs=1, space="DRAM"))

            ident = consts.tile([P, P], dt.float32)
            masks.make_identity(nc, ident[:])
            identB = consts.tile([P, P], at_dtype)
            masks.make_identity(nc, identB[:])

            ones1 = consts.tile([1, P], dt.float32)
            nc.gpsimd.memset(ones1[:], 1.0)

            # local degree columns: col i = row sums of local m-tile i
            deg_col = deg_pool.tile([P, MT], dt.float32)
            nc.gpsimd.memset(deg_col[:], 0.0)  # garbage lanes stay rsqrt-safe
            dgT = deg_pool.tile([P, P], dt.float32, tag="dgT")
            nc.gpsimd.memset(dgT[:KT, :], 1.0)

            # theta is a single-descriptor HWDGE load; everything else with
            # rearranged (many-small-descriptor) patterns goes on SWDGE --
            # HWDGE descriptor generation for those blocks its issuing
            # engine's queue for ~18us (measured), which starved the ACT
            # rowsums in v5.
            theta_sb = consts.tile([F, F], dt.float32)
            nc.sync.dma_start(theta_sb[:], theta[:])
            thetaB = consts.tile([F, F], dt.float32)
            nc.vector.tensor_scalar_mul(thetaB[:], theta_sb[:], beta)

            xg = xs_pool.tile([P, KT * F], dt.float32)
            xn_all = xs_pool.tile([P, MT * F], dt.float32, tag="xn_all")
            hn_all = xs_pool.tile([P, MT * F], dt.float32, tag="hn_all")

            def emit_xghn_loads():
                # SWDGE, emitted AFTER the adj pair-loads so they queue
                # behind the critical stream
                if kfull:
                    nc.gpsimd.dma_start(
                        xg[:].rearrange("p (kb f) -> p kb f", kb=KT)[:, 0:kfull, :],
                        x_full[0 : kfull * P, :].rearrange("(kb p) f -> p kb f", p=P),
                    )
                if ktail:
                    nc.gpsimd.dma_start(
                        xg[0:ktail, kfull * F : (kfull + 1) * F],
                        x_full[kfull * P : n_nodes, :],
                    )
                for src, dst in ((x_loc, xn_all), (h0_loc, hn_all)):
                    if mfull:
                        nc.gpsimd.dma_start(
                            dst[:].rearrange("p (i f) -> p i f", i=MT)[:, 0:mfull, :],
                            src[0 : mfull * P, :].rearrange("(i p) f -> p i f", p=P),
                        )
                    if mtail:
                        nc.gpsimd.dma_start(
                            dst[0:mtail, mfull * F : (mfull + 1) * F],
                            src[mfull * P : m_rows, :],
                        )

            # A^T resident in SBUF: [k_local, (m_tile, kb, m_local)] -- one
            # contiguous [KT, 128] region per m-tile so the blocked xbar
            # transpose writes it in a single instruction
            AT = at_pool.tile([P, MT * KTP], at_dtype)
            AT4 = AT[:].rearrange("p (i kb m) -> p i kb m", i=MT, kb=KT)

            xT = xs_pool.tile([P, m_rows], dt.float32, tag="xT")
            h0aT = xs_pool.tile([P, m_rows], dt.float32, tag="h0aT")

            # ---------------- Phase 1: stream adj, rowsum + transpose ----------
            # PAIR granularity loads: one SWDGE cast-load (fp32->bf16) covers
            # two m-tiles.  NO xbar: the DMA-transpose corrupts data when two
            # of them overlap on the two HWDGE rings, and Tile serializes
            # every SWDGE DMA against in-flight xbars anyway.  Instead the
            # (otherwise idle) PE transposes A in [128,128] blocks, 4 blocks
            # into one PSUM bank, and DVE drains each bank with a single
            # 512-wide copy into bf16 A^T.  ACT does the rowsums (Copy with
            # accum_out into a throwaway scratch so the PE transposes never
            # serialize behind it).  Every access is visible to Tile's dep
            # tracker -- no manual fences anywhere in the pipeline.
            pairs = [(j, list(range(2 * j, min(2 * j + 2, MT))))
                     for j in range((MT + 1) // 2)]
            KB_GRP = 4              # transposed blocks per PSUM bank drain
            xs = xs_pool.tile([P, KT * F], at_dtype, tag="xs")
            red_scr = xs  # reduce scratch aliases xs (rewritten in phase 2)
            bank_ctx = tc.tile_pool(name="bank_ps", bufs=3, space="PSUM")
            bank_pool = bank_ctx.__enter__()

            # Early PE warm-up so phase-1 transposes start at 2.4 GHz
            # (overlaps the first loads; PE is idle anyway)
            with tc.tile_pool(name="warm0_ps", bufs=1, space="PSUM") as w0_pool:
                wp0 = w0_pool.tile([P, P], dt.float32)
                for j in range(N_WARM):
                    nc.tensor.matmul(
                        wp0[:P, 0:P], theta_sb[:F, :F], theta_sb[:F, :F],
                        start=(j == 0), stop=(j == N_WARM - 1),
                    )

            nat_slots = {}

            def emit_pair_load(j, tiles):
                nat = stream_pool.tile([P, 2 * KTP], at_dtype, tag="nat")
                nat_slots[j] = nat
                nat2 = nat[:].rearrange("p (t k) -> p t k", t=2)
                if len(tiles) == 2 and mh[tiles[0]] == P and mh[tiles[1]] == P:
                    nc.gpsimd.dma_start(
                        nat2[:, :, 0:n_nodes],
                        adj[2 * P * j : 2 * P * (j + 1), :]
                        .rearrange("(t p) k -> p t k", p=P),
                    )
                else:
                    for s, i in enumerate(tiles):
                        h = mh[i]
                        nc.gpsimd.dma_start(
                            nat2[:h, s, 0:n_nodes],
                            adj[P * i : P * i + h, :],
                        )

            def emit_pair_reduces(j, tiles):
                nat2 = nat_slots[j][:].rearrange("p (t k) -> p t k", t=2)
                for s, i in enumerate(tiles):
                    h = mh[i]
                    # rowsum on ACT (fp32 accumulator), output discarded
                    nc.scalar.activation(
                        red_scr[:h, 0:n_nodes], nat2[:h, s, 0:n_nodes],
                        AF.Copy, accum_out=deg_col[:h, i : i + 1],
                    )

            # HAM heartbeat: PE transpose-mode activity does NOT register as
            # busy for the clock gate, so sprinkle tiny real matmuls between
            # transpose groups to hold the 2.4 GHz clock (else the whole
            # transpose stream runs at 1.2).
            hb = ptx_pool.tile([P, 64], dt.float32, tag="hb")

            def emit_heartbeat():
                nc.tensor.matmul(
                    hb[:P, 0:64], theta_sb[:F, :F], theta_sb[:F, 0:64],
                    start=True, stop=True,
                )

            def emit_pair_at(j, tiles):
                # A^T blocks via PE; drain 4 blocks per DVE copy
                nat = nat_slots[j]
                for s, i in enumerate(tiles):
                    for kb0 in range(0, KT, KB_GRP):
                        kbn = min(KB_GRP, KT - kb0)
                        bank = bank_pool.tile([P, KB_GRP * P], at_dtype,
                                              tag="tb")
                        for kk in range(kbn):
                            c0 = s * KTP + (kb0 + kk) * P
                            nc.tensor.transpose(
                                bank[:P, kk * P : kk * P + P],
                                nat[:P, c0 : c0 + P],
                                identB[:P, :P],
                            )
                        nc.vector.tensor_copy(
                            AT4[:, i, kb0 : kb0 + kbn, :],
                            bank[:P, 0 : kbn * P],
                        )
                        if (kb0 // KB_GRP) % 3 == 2:
                            emit_heartbeat()
                    emit_heartbeat()

            for j, tiles in pairs[:-1]:
                emit_pair_load(j, tiles)
                emit_pair_reduces(j, tiles)
                emit_pair_at(j, tiles)
            jL, tilesL = pairs[-1]
            emit_pair_load(jL, tilesL)
            emit_xghn_loads()
            emit_pair_reduces(jL, tilesL)
            # deg tail is emitted BEFORE the last pair's A^T transposes so
            # the dis stores + AllGather dispatch ~8us earlier; the last A^T
            # blocks transpose during the collective.

            # ---------------- degree tail: dis = rsqrt(deg+1), exchange -------
            degp = deg_pool.tile([P, MT], dt.float32, tag="degp")
            nc.vector.tensor_scalar_add(degp[:], deg_col[:], 1.0)
            dis_col = deg_pool.tile([P, MT], dt.float32, tag="dis_col")
            nc.vector.reciprocal(dis_col[:], degp[:])
            nc.scalar.sqrt(dis_col[:], dis_col[:])

            degT_ps = ptx_pool.tile([P, P], dt.float32, tag="sm")
            nc.tensor.transpose(degT_ps[:MT, :P], dis_col[:P, :MT], ident[:P, :P])
            disT = deg_pool.tile([P, P], dt.float32, tag="disT")
            nc.vector.tensor_copy(disT[:MT, :P], degT_ps[:MT, :P])

            dis_loc_d = dram.tile([m_rows], dt.float32)
            dis_full_d = dram.tile([n_nodes], dt.float32)
            st_insts = []
            if mfull:
                st_insts.append(nc.gpsimd.dma_start(
                    dis_loc_d[0 : mfull * P].rearrange("(a b) -> a b", b=P),
                    disT[0:mfull, :],
                ))
            if mtail:
                st_insts.append(nc.gpsimd.dma_start(
                    dis_loc_d[mfull * P : m_rows].rearrange("(a b) -> a b", a=1),
                    disT[mfull : mfull + 1, 0:mtail],
                ))
            # local dis row reload happens BEFORE the collective on the
            # gpsimd FIFO so qT prep can overlap the AllGather
            rs_b = tvec_pool.tile([P, m_rows], dt.float32, tag="rs_b")
            dis_row = rs_b[0:1, :]
            ld_row = nc.gpsimd.dma_start(
                dis_row[0:1, 0:m_rows],
                dis_loc_d[:].rearrange("(a b) -> a b", a=1),
            )
            for st in st_insts:
                _adh(ld_row.ins, st.ins, sync=True,
                     reason="dis row reload after both dis stores")
            # Explicitly fence the collective's READ of dis_loc_d on both
            # stores' completion: the DRAM-tile dep does not reliably hold
            # the CC stream's read (v3 corrupted the last dis elements when
            # the AG fired right behind the tail store).
            ag = nc.gpsimd.collective_compute(
                "AllGather",
                mybir.AluOpType.bypass,
                replica_groups=groups,
                ins=[dis_loc_d[:]],
                outs=[dis_full_d[:]],
            )
            for st in st_insts:
                _adh(ag.ins, st.ins, sync=True,
                     reason="collective reads dis_loc_d after both stores")

            # last pair's A^T transposes + the x/h0 epilogue transposes run
            # during the collective
            emit_pair_at(jL, tilesL)
            for i in range(MT):
                h = mh[i]
                xt_ps = ptx_pool.tile([P, P], dt.float32, tag="sm")
                nc.tensor.transpose(
                    xt_ps[:F, :h], xn_all[:h, i * F : i * F + F], ident[:h, :h]
                )
                nc.vector.tensor_copy(xT[:, P * i : P * i + h], xt_ps[:F, :h])
                ht_ps = ptx_pool.tile([P, P], dt.float32, tag="sm")
                nc.tensor.transpose(
                    ht_ps[:F, :h], hn_all[:h, i * F : i * F + F], ident[:h, :h]
                )
                nc.scalar.activation(
                    h0aT[:, P * i : P * i + h], ht_ps[:F, :h], AF.Copy,
                    scale=alpha,
                )
            bank_ctx.__exit__(None, None, None)

            # broadcast dis across partitions via PE ones-matmuls (keeps the
            # gpsimd FIFO free so the AllGather dispatches immediately), then
            # rs = c1*dis, s1 = c1*dis^2, qT = s1*x^T + alpha*h0^T -- all
            # overlapping the collective.
            s1_b = tvec_pool.tile([P, m_rows], dt.float32, tag="s1_b")
            qT = xs_pool.tile([P, m_rows], dt.float32, tag="qT")
            with tc.tile_pool(name="bc_ps", bufs=2, space="PSUM") as bc_pool:
                for s in range(0, m_rows, CHUNK):
                    wc = min(CHUNK, m_rows - s)
                    bc = bc_pool.tile([P, CHUNK], dt.float32)
                    nc.tensor.matmul(
                        bc[:P, 0:wc], ones1[0:1, :P], dis_row[0:1, s : s + wc],
                        start=True, stop=True,
                    )
                    nc.vector.tensor_copy(s1_b[:, s : s + wc], bc[:P, 0:wc])
            nc.vector.tensor_scalar_mul(rs_b[:], s1_b[:], c1)
            nc.vector.tensor_mul(s1_b[:], s1_b[:], rs_b[:])
            nc.vector.tensor_mul(qT[:], xT[:], s1_b[:])
            nc.vector.tensor_add(qT[:], qT[:], h0aT[:])

            # PE warm-up matmuls spanning the collective so phase-2 matmuls
            # start at the warm 2.4 GHz clock (fp32 on thetaB, no deps)
            with tc.tile_pool(name="warm_ps", bufs=1, space="PSUM") as warm_pool:
                wp = warm_pool.tile([P, P], dt.float32)
                for j in range(N_WARM):
                    nc.tensor.matmul(
                        wp[:P, 0:P],
                        theta_sb[:F, :F],
                        theta_sb[:F, :F],
                        start=(j == 0),
                        stop=(j == N_WARM - 1),
                    )

            # global dis -> per k-block column layout [P, KT]
            if kfull:
                dg_ld = nc.gpsimd.dma_start(
                    dgT[0:kfull, 0:P],
                    dis_full_d[0 : kfull * P].rearrange("(a b) -> a b", b=P),
                )
                _adh(dg_ld.ins, ag.ins, sync=True,
                     reason="dis_full reload after collective completes")
            if ktail:
                dg_ld = nc.gpsimd.dma_start(
                    dgT[kfull : kfull + 1, 0:ktail],
                    dis_full_d[kfull * P : n_nodes],
                )
                _adh(dg_ld.ins, ag.ins, sync=True,
                     reason="dis_full reload after collective completes")
            dg_ps = ptx_pool.tile([P, P], dt.float32, tag="sm")
            nc.tensor.transpose(dg_ps[:P, :KT], dgT[:KT, :P], ident[:KT, :KT])
            disg = deg_pool.tile([P, KT], dt.float32, tag="disg")
            nc.vector.tensor_copy(disg[:], dg_ps[:P, :KT])

            if debug_dump:
                dbg_at = nc.dram_tensor(
                    "dbg_at", [P, MT * KTP], at_dtype, kind="ExternalOutput"
                )
                nc.sync.dma_start(dbg_at[:], AT[:])
                dbg_deg = nc.dram_tensor(
                    "dbg_deg", [P, MT], dt.float32, kind="ExternalOutput"
                )
                nc.sync.dma_start(dbg_deg[:], deg_col[:])
                dbg_disg = nc.dram_tensor(
                    "dbg_disg", [P, P], dt.float32, kind="ExternalOutput"
                )
                nc.sync.dma_start(dbg_disg[:], dgT[:])

            # ---------------- Phase 2: xs scaling + matmuls + epilogue --------
            # xs = D^-1/2 x in bf16; the scale ops stream on DVE just ahead
            # of the chunk matmuls on PE.
            for kb in range(KT):
                w = kw[kb]
                nc.vector.tensor_scalar_mul(
                    xs[:w, kb * F : kb * F + F],
                    xg[:w, kb * F : kb * F + F],
                    disg[:w, kb : kb + 1],
                )

            out_sb = xs_pool.tile([P, MT * F], dt.float32, tag="out_sb")
            with tc.tile_pool(name="hi_ps", bufs=2, space="PSUM") as hi_pool, \
                 tc.tile_pool(name="o2_ps", bufs=2, space="PSUM") as o2_pool:
                hi_tiles = []

                def emit_mms(ci):
                    s, wc, ia, ib, tw = mchunks[ci]
                    hiT = hi_pool.tile([P, CHUNK], dt.float32)
                    for kb in range(KT):
                        w = kw[kb]
                        nc.tensor.matmul(
                            hiT[:F, 0:wc],
                            xs[:w, kb * F : kb * F + F],
                            AT4[:w, ia:ib, kb, 0:tw],
                            start=(kb == 0),
                            stop=(kb == KT - 1),
                        )
                    hi_tiles.append(hiT)

                def emit_epilogue(ci):
                    s, wc, ia, ib, tw = mchunks[ci]
                    hiT = hi_tiles[ci]
                    supT = sup_pool.tile([P, CHUNK], dt.float32)
                    nc.vector.tensor_mul(
                        supT[:, 0:wc], hiT[:F, 0:wc], rs_b[:, s : s + wc]
                    )
                    nc.vector.tensor_add(
                        supT[:, 0:wc], supT[:, 0:wc], qT[:, s : s + wc]
                    )
                    o2T = o2_pool.tile([P, CHUNK], dt.float32)
                    nc.tensor.matmul(
                        o2T[:F, 0:wc], thetaB[:F, :F], supT[:F, 0:wc],
                        start=True, stop=True,
                    )
                    outT = outc_pool.tile([P, CHUNK], dt.float32)
                    nc.vector.scalar_tensor_tensor(
                        outT[:, 0:wc], supT[:, 0:wc], 1.0 - beta, o2T[:F, 0:wc],
                        mybir.AluOpType.mult, mybir.AluOpType.add,
                    )
                    # back to natural [m, f] into the staging tile
                    for off in range(0, wc, P):
                        hh = min(P, wc - off)
                        ti_ = ia + off // P
                        ot_ps = ptx_pool.tile([P, P], dt.float32, tag="sm")
                        nc.tensor.transpose(
                            ot_ps[:hh, :F], outT[:F, off : off + hh], ident[:F, :F]
                        )
                        nc.vector.tensor_copy(
                            out_sb[:hh, ti_ * F : ti_ * F + F], ot_ps[:hh, :F]
                        )
                    # store this chunk's rows
                    if tw == P:
                        nc.gpsimd.dma_start(
                            out_d[s : s + wc, :].rearrange("(i p) f -> p i f", p=P),
                            out_sb[:].rearrange("p (i f) -> p i f", i=MT)[:, ia:ib, :],
                        )
                    else:
                        nc.gpsimd.dma_start(
                            out_d[s : s + wc, :], out_sb[0:wc, ia * F : ia * F + F]
                        )

                emit_mms(0)
                for ci in range(1, len(mchunks)):
                    emit_mms(ci)
                    emit_epilogue(ci - 1)
                emit_epilogue(len(mchunks) - 1)

    nc.compile()
    return nc


def make_in_maps(x, adj, h0, theta, n_cores):
    m = x.shape[1] // 2
    in_maps = []
    for c in range(n_cores):
        b, half = c // 2, c % 2
        r0 = half * m
        in_maps.append(
            {
                "adj_rows": adj[b, r0 : r0 + m, :],
                "x_full": x[b],
                "x_loc": x[b, r0 : r0 + m, :],
                "h0_loc": h0[b, r0 : r0 + m, :],
                "theta": theta,
            }
        )
    return in_maps


_CACHE = {}


def _get_program(key, *args, **kwargs):
    if key not in _CACHE:
        _CACHE[key] = build_program(*args, **kwargs)
    return _CACHE[key]


def kernel(x, adj, h0, theta, lamda, alpha, l):
    x = np.asarray(x, dtype=np.float32)
    adj = np.asarray(adj, dtype=np.float32)
    h0 = np.asarray(h0, dtype=np.float32)
    theta = np.asarray(theta, dtype=np.float32)
    lamda_f = float(np.asarray(lamda))
    alpha_f = float(np.asarray(alpha))
    l_f = float(np.asarray(l))
    beta_f = float(math.log(lamda_f / l_f + 1.0))

    B, N, Fdim = x.shape
    assert (B, N, Fdim) == (B_FULL, N_FULL, F)
    M = N // 2

    nc = _get_program(
        ("full", alpha_f, beta_f), N, M, N_CORES_FULL, alpha_f, beta_f
    )

    in_maps = make_in_maps(x, adj, h0, theta, N_CORES_FULL)
    res = bass_utils.run_bass_kernel_spmd(
        nc, in_maps, list(range(N_CORES_FULL))
    ).results

    out = np.empty((B, N, Fdim), dtype=np.float32)
    for c in range(N_CORES_FULL):
        b, half = c // 2, c % 2
        out[b, half * M : (half + 1) * M, :] = res[c]["out"]
    return out
